# revision 8
# baseline (speedup 1.0000x reference)
"""CenterLoss on 8 Trainium2 NeuronCores (Bass/Tile).

loss = clip(distmat * onehot(labels), 1e-12, 1e12).sum() / B
     = (sum_i clip(||x_i - c_{y_i}||^2, 1e-12, 1e12) + B*(C-1)*1e-12) / B

Data-parallel over the batch: each core gets 4096 rows of x/labels plus
the replicated centers table (SBUF-resident).  The per-sample center
gather c_{y_i} is split across two engines that run concurrently,
interleaved within every 4-tile group so neither path head-of-line
blocks the shared vector/scalar pipeline:

  - first (4 - K_PE) tiles of each group: GpSimd indirect DMA (SWDGE
    descriptor generation, ~1.3us per 128 rows)
  - last K_PE tiles: TensorEngine onehot-matmul gather — G_tile =
    onehot[128s x 1024c] @ centers[1024c x 256f] as 8 accumulating
    128-wide matmuls into PSUM.  Exact in fp32 (onehot rows select one
    center row).  The group's onehot slab [k=128, chunk=8, s] is built
    in ONE vector-engine op: is_equal(labels[s], k + 128q) with
    broadcast access patterns.

Per tile the vector engine computes x - c and the scalar engine squares
with a fused per-sample row-sum.  Distances are clipped on-device;
per-core partial scalars are summed on the host (the sanctioned
all-reduce).
"""

import numpy as np

BATCH, NUM_CLASSES, FEATURE_DIM = 32768, 1024, 256
N_CORES = 8
SHARD = BATCH // N_CORES  # 4096
P = 128
N_TILES = SHARD // P  # 32
GROUP = 4  # tiles per x-DMA / onehot-build group
N_GROUPS = N_TILES // GROUP
N_CHUNKS = NUM_CLASSES // P  # 8 class chunks
CLAMP_MIN, CLAMP_MAX = 1e-12, 1e12

K_PE = 4  # last K_PE tiles of each 4-tile group gather via TensorEngine

_CACHE: dict = {}


def _build_nc():
    import concourse.bacc as bacc
    import concourse.bass as bass
    import concourse.tile as tile
    from concourse import mybir

    f32 = mybir.dt.float32
    i32 = mybir.dt.int32

    nc = bacc.Bacc("TRN2", target_bir_lowering=False, debug=False)

    x_d = nc.dram_tensor("x", [SHARD, FEATURE_DIM], f32, kind="ExternalInput")
    # labels pre-transposed on host to [P, N_TILES]: lab[p, t] = labels[t*P + p]
    lab_d = nc.dram_tensor("labels", [P, N_TILES], i32, kind="ExternalInput")
    # labels broadcast to all partitions as f32: labT[p, i] = labels[i]
    labt_d = nc.dram_tensor("labt", [P, SHARD], f32, kind="ExternalInput")
    # patt[p, q] = p + 128*q
    patt_d = nc.dram_tensor("patt", [P, N_CHUNKS], f32, kind="ExternalInput")
    cen_d = nc.dram_tensor(
        "centers", [NUM_CLASSES, FEATURE_DIM], f32, kind="ExternalInput"
    )
    out_d = nc.dram_tensor("out", [1, 1], f32, kind="ExternalOutput")

    with tile.TileContext(nc) as tc:
        with (
            tc.tile_pool(name="data", bufs=3) as data,
            tc.tile_pool(name="gbuf", bufs=6) as gbuf,
            tc.tile_pool(name="oh", bufs=3) as ohp,
            tc.tile_pool(name="work", bufs=6) as work,
            tc.tile_pool(name="single", bufs=1) as single,
            tc.tile_pool(name="psumg", bufs=6, space="PSUM") as psumg,
            tc.tile_pool(name="psum1", bufs=1, space="PSUM") as psum1,
        ):
            lab_all = single.tile([P, N_TILES], i32)
            nc.sync.dma_start(out=lab_all[:], in_=lab_d[:, :])
            patt_t = single.tile([P, N_CHUNKS], f32)
            nc.sync.dma_start(out=patt_t[:], in_=patt_d[:, :])
            labt_all = single.tile([P, SHARD], f32)
            nc.sync.dma_start(out=labt_all[:], in_=labt_d[:, :])
            cen_sb = single.tile([P, N_CHUNKS, FEATURE_DIM], f32)
            nc.sync.dma_start(
                out=cen_sb[:],
                in_=cen_d[:, :].rearrange("(q k) e -> k q e", k=P),
            )

            acc = single.tile([P, N_TILES], f32)
            for g in range(N_GROUPS):
                x_t = data.tile([P, GROUP, FEATURE_DIM], f32, tag="x")
                nc.sync.dma_start(
                    out=x_t[:],
                    in_=x_d[g * GROUP * P : (g + 1) * GROUP * P, :].rearrange(
                        "(t p) e -> p t e", p=P
                    ),
                )
                oh_t = None
                if K_PE:
                    # onehot slab for this group's PE tiles in one DVE op:
                    # oh[k, q, s] = (labels[s] == k + 128q)
                    base = (g * GROUP + (GROUP - K_PE)) * P
                    w = K_PE * P
                    oh_t = ohp.tile([P, N_CHUNKS, w], f32, tag="oh")
                    nc.vector.tensor_tensor(
                        out=oh_t[:],
                        in0=labt_all[:, base : base + w]
                        .rearrange("p (o s) -> p o s", o=1)
                        .to_broadcast([P, N_CHUNKS, w]),
                        in1=patt_t[:]
                        .rearrange("p (q o) -> p q o", o=1)
                        .to_broadcast([P, N_CHUNKS, w]),
                        op=mybir.AluOpType.is_equal,
                    )
                for j in range(GROUP):
                    t = g * GROUP + j
                    if j < GROUP - K_PE:
                        g_t = gbuf.tile([P, FEATURE_DIM], f32, tag="g")
                        nc.gpsimd.indirect_dma_start(
                            out=g_t[:],
                            out_offset=None,
                            in_=cen_d[:, :],
                            in_offset=bass.IndirectOffsetOnAxis(
                                ap=lab_all[:, t : t + 1], axis=0
                            ),
                        )
                        gather_src = g_t[:]
                    else:
                        jj = j - (GROUP - K_PE)
                        g_ps = psumg.tile([P, FEATURE_DIM], f32, tag="gps")
                        for q in range(N_CHUNKS):
                            nc.tensor.matmul(
                                out=g_ps[:],
                                lhsT=oh_t[:, q, jj * P : (jj + 1) * P],
                                rhs=cen_sb[:, q, :],
                                start=(q == 0),
                                stop=(q == N_CHUNKS - 1),
                            )
                        gather_src = g_ps[:]
                    d_t = work.tile([P, FEATURE_DIM], f32, tag="d")
                    nc.vector.tensor_tensor(
                        out=d_t[:],
                        in0=x_t[:, j, :],
                        in1=gather_src,
                        op=mybir.AluOpType.subtract,
                    )
                    s_t = work.tile([P, FEATURE_DIM], f32, tag="s")
                    nc.scalar.activation(
                        out=s_t[:],
                        in_=d_t[:],
                        func=mybir.ActivationFunctionType.Square,
                        accum_out=acc[:, t : t + 1],
                    )

            clipped = single.tile([P, N_TILES], f32)
            nc.vector.tensor_scalar(
                out=clipped[:],
                in0=acc[:],
                scalar1=float(CLAMP_MIN),
                scalar2=float(CLAMP_MAX),
                op0=mybir.AluOpType.max,
                op1=mybir.AluOpType.min,
            )
            rowsum = single.tile([P, 1], f32)
            nc.vector.reduce_sum(out=rowsum[:], in_=clipped[:], axis=mybir.AxisListType.X)

            ones = single.tile([P, 1], f32)
            nc.vector.memset(ones[:], 1.0)
            tot = psum1.tile([1, 1], f32, space="PSUM")
            nc.tensor.matmul(out=tot[:], lhsT=rowsum[:], rhs=ones[:], start=True, stop=True)
            res = single.tile([1, 1], f32)
            nc.vector.tensor_copy(out=res[:], in_=tot[:])
            nc.sync.dma_start(out=out_d[:, :], in_=res[:])

    nc.finalize()
    return nc


def kernel(x: np.ndarray, centers: np.ndarray, labels: np.ndarray) -> np.ndarray:
    from concourse import bass_utils

    if "nc" not in _CACHE:
        _CACHE["nc"] = _build_nc()
    nc = _CACHE["nc"]

    x = np.ascontiguousarray(np.asarray(x, dtype=np.float32))
    centers = np.ascontiguousarray(np.asarray(centers, dtype=np.float32))
    lab64 = np.asarray(labels).astype(np.int64).reshape(N_CORES, SHARD)
    patt = (
        np.arange(P, dtype=np.float32)[:, None]
        + 128.0 * np.arange(N_CHUNKS, dtype=np.float32)[None, :]
    )

    xs = x.reshape(N_CORES, SHARD, FEATURE_DIM)
    in_maps = []
    for c in range(N_CORES):
        labc = lab64[c]
        in_maps.append({
            "x": np.ascontiguousarray(xs[c]),
            "labels": np.ascontiguousarray(
                labc.reshape(N_TILES, P).T.astype(np.int32)
            ),
            "labt": np.ascontiguousarray(
                np.broadcast_to(labc.astype(np.float32), (P, SHARD))
            ),
            "patt": np.ascontiguousarray(patt),
            "centers": centers,
        })

    rr = bass_utils.run_bass_kernel_spmd(nc, in_maps, list(range(N_CORES)))
    _CACHE["last_results"] = rr

    total = sum(float(r["out"][0, 0]) for r in rr.results)
    loss = (total + BATCH * (NUM_CLASSES - 1) * CLAMP_MIN) / BATCH
    return np.asarray(loss, dtype=np.float32)


# revision 9
# speedup vs baseline: 2.2429x; 2.2429x over previous
"""CenterLoss on 8 Trainium2 NeuronCores (Bass/Tile).

loss = clip(distmat * onehot(labels), 1e-12, 1e12).sum() / B
     = (sum_i clip(||x_i - c_{y_i}||^2, 1e-12, 1e12) + B*(C-1)*1e-12) / B

Data-parallel over the batch: each of the 8 cores gets 4096 rows of x and
labels plus the replicated centers table.  x streams in via 4 big DMAs
(1MB each, issued upfront); the label-selected center rows are fetched
128 at a time with indirect DMAs — the GpSimd SWDGE descriptor
generation (~1.3us per 128 rows) is the critical path, so every other
engine's work is sized to hide underneath it: per 128-row tile the
vector engine computes x-c and the scalar engine squares with a fused
per-sample row-sum.  Per-sample distances are clipped on-device; the 8
per-core partial scalars are summed on the host (the sanctioned
all-reduce).

Notes from profiling on trn2: a multi-column offset AP on
indirect_dma_start corrupts data (descriptor/dest zip mismatch); the
dma_gather custom ucode is no faster per row and costs a ~20us library
load; an exact onehot-matmul gather on the TensorEngine runs ~3x slower
than the SWDGE path (LDWEIGHTS can't hide behind same-bank accumulating
matmuls).  Hence the all-SWDGE design with deep buffering.
"""

import numpy as np

BATCH, NUM_CLASSES, FEATURE_DIM = 32768, 1024, 256
N_CORES = 8
SHARD = BATCH // N_CORES  # 4096
P = 128
N_TILES = SHARD // P  # 32
GROUP = 8  # tiles per x-DMA
N_GROUPS = N_TILES // GROUP
CLAMP_MIN, CLAMP_MAX = 1e-12, 1e12

_CACHE: dict = {}


def _build_nc():
    import concourse.bacc as bacc
    import concourse.bass as bass
    import concourse.tile as tile
    from concourse import mybir

    f32 = mybir.dt.float32
    i32 = mybir.dt.int32

    nc = bacc.Bacc("TRN2", target_bir_lowering=False, debug=False)

    x_d = nc.dram_tensor("x", [SHARD, FEATURE_DIM], f32, kind="ExternalInput")
    # labels pre-transposed on host to [P, N_TILES]: lab[p, t] = labels[t*P + p]
    lab_d = nc.dram_tensor("labels", [P, N_TILES], i32, kind="ExternalInput")
    cen_d = nc.dram_tensor(
        "centers", [NUM_CLASSES, FEATURE_DIM], f32, kind="ExternalInput"
    )
    out_d = nc.dram_tensor("out", [1, 1], f32, kind="ExternalOutput")

    with tile.TileContext(nc) as tc:
        with (
            tc.tile_pool(name="data", bufs=N_GROUPS) as data,
            tc.tile_pool(name="gbuf", bufs=16) as gbuf,
            tc.tile_pool(name="work", bufs=8) as work,
            tc.tile_pool(name="single", bufs=1) as single,
            tc.tile_pool(name="psum", bufs=1, space="PSUM") as psum,
        ):
            lab_all = single.tile([P, N_TILES], i32)
            nc.sync.dma_start(out=lab_all[:], in_=lab_d[:, :])

            # all x group-DMAs issued upfront (bufs == N_GROUPS)
            x_tiles = []
            for g in range(N_GROUPS):
                x_t = data.tile([P, GROUP, FEATURE_DIM], f32, tag="x")
                nc.sync.dma_start(
                    out=x_t[:],
                    in_=x_d[g * GROUP * P : (g + 1) * GROUP * P, :].rearrange(
                        "(t p) e -> p t e", p=P
                    ),
                )
                x_tiles.append(x_t)

            acc = single.tile([P, N_TILES], f32)
            for t in range(N_TILES):
                g, j = divmod(t, GROUP)
                g_t = gbuf.tile([P, FEATURE_DIM], f32, tag="g")
                nc.gpsimd.indirect_dma_start(
                    out=g_t[:],
                    out_offset=None,
                    in_=cen_d[:, :],
                    in_offset=bass.IndirectOffsetOnAxis(
                        ap=lab_all[:, t : t + 1], axis=0
                    ),
                )
                d_t = work.tile([P, FEATURE_DIM], f32, tag="d")
                nc.vector.tensor_tensor(
                    out=d_t[:],
                    in0=x_tiles[g][:, j, :],
                    in1=g_t[:],
                    op=mybir.AluOpType.subtract,
                )
                s_t = work.tile([P, FEATURE_DIM], f32, tag="s")
                nc.scalar.activation(
                    out=s_t[:],
                    in_=d_t[:],
                    func=mybir.ActivationFunctionType.Square,
                    accum_out=acc[:, t : t + 1],
                )

            clipped = single.tile([P, N_TILES], f32)
            nc.vector.tensor_scalar(
                out=clipped[:],
                in0=acc[:],
                scalar1=float(CLAMP_MIN),
                scalar2=float(CLAMP_MAX),
                op0=mybir.AluOpType.max,
                op1=mybir.AluOpType.min,
            )
            rowsum = single.tile([P, 1], f32)
            nc.vector.reduce_sum(out=rowsum[:], in_=clipped[:], axis=mybir.AxisListType.X)

            ones = single.tile([P, 1], f32)
            nc.vector.memset(ones[:], 1.0)
            tot = psum.tile([1, 1], f32, space="PSUM")
            nc.tensor.matmul(out=tot[:], lhsT=rowsum[:], rhs=ones[:], start=True, stop=True)
            res = single.tile([1, 1], f32)
            nc.vector.tensor_copy(out=res[:], in_=tot[:])
            nc.sync.dma_start(out=out_d[:, :], in_=res[:])

    nc.finalize()
    return nc


def kernel(x: np.ndarray, centers: np.ndarray, labels: np.ndarray) -> np.ndarray:
    from concourse import bass_utils

    if "nc" not in _CACHE:
        _CACHE["nc"] = _build_nc()
    nc = _CACHE["nc"]

    x = np.ascontiguousarray(np.asarray(x, dtype=np.float32))
    centers = np.ascontiguousarray(np.asarray(centers, dtype=np.float32))
    lab = np.asarray(labels).astype(np.int64).reshape(N_CORES, N_TILES, P)

    xs = x.reshape(N_CORES, SHARD, FEATURE_DIM)
    in_maps = [
        {
            "x": np.ascontiguousarray(xs[c]),
            "labels": np.ascontiguousarray(lab[c].transpose(1, 0).astype(np.int32)),
            "centers": centers,
        }
        for c in range(N_CORES)
    ]

    rr = bass_utils.run_bass_kernel_spmd(nc, in_maps, list(range(N_CORES)))
    _CACHE["last_results"] = rr

    total = sum(float(r["out"][0, 0]) for r in rr.results)
    loss = (total + BATCH * (NUM_CLASSES - 1) * CLAMP_MIN) / BATCH
    return np.asarray(loss, dtype=np.float32)


# revision 10
# speedup vs baseline: 2.2474x; 1.0020x over previous
"""CenterLoss on 8 Trainium2 NeuronCores (Bass/Tile).

loss = clip(distmat * onehot(labels), 1e-12, 1e12).sum() / B
     = (sum_i clip(||x_i - c_{y_i}||^2, 1e-12, 1e12) + B*(C-1)*1e-12) / B

Data-parallel over the batch: each of the 8 cores gets 4096 rows of x and
labels plus the replicated centers table.  x streams in via 4 big DMAs
(1MB each, issued upfront); the label-selected center rows are fetched
128 at a time with indirect DMAs — the GpSimd SWDGE descriptor
generation (~1.3us per 128 rows) is the critical path, so every other
engine's work is sized to hide underneath it: per 128-row tile the
vector engine computes x-c and the scalar engine squares with a fused
per-sample row-sum.  Per-sample distances are clipped on-device; the 8
per-core partial scalars are summed on the host (the sanctioned
all-reduce).

Notes from profiling on trn2: a multi-column offset AP on
indirect_dma_start corrupts data (descriptor/dest zip mismatch); the
dma_gather custom ucode is no faster per row and costs a ~20us library
load; an exact onehot-matmul gather on the TensorEngine runs ~3x slower
than the SWDGE path (LDWEIGHTS can't hide behind same-bank accumulating
matmuls).  Hence the all-SWDGE design with deep buffering.
"""

import numpy as np

BATCH, NUM_CLASSES, FEATURE_DIM = 32768, 1024, 256
N_CORES = 8
SHARD = BATCH // N_CORES  # 4096
P = 128
N_TILES = SHARD // P  # 32
GROUP = 8  # tiles per x-DMA
N_GROUPS = N_TILES // GROUP
CLAMP_MIN, CLAMP_MAX = 1e-12, 1e12

_CACHE: dict = {}


def _build_nc():
    import concourse.bacc as bacc
    import concourse.bass as bass
    import concourse.tile as tile
    from concourse import mybir

    f32 = mybir.dt.float32
    i32 = mybir.dt.int32

    nc = bacc.Bacc("TRN2", target_bir_lowering=False, debug=False)

    x_d = nc.dram_tensor("x", [SHARD, FEATURE_DIM], f32, kind="ExternalInput")
    # labels pre-transposed on host to [P, N_TILES]: lab[p, t] = labels[t*P + p]
    lab_d = nc.dram_tensor("labels", [P, N_TILES], i32, kind="ExternalInput")
    cen_d = nc.dram_tensor(
        "centers", [NUM_CLASSES, FEATURE_DIM], f32, kind="ExternalInput"
    )
    out_d = nc.dram_tensor("out", [1, 1], f32, kind="ExternalOutput")

    with tile.TileContext(nc) as tc:
        with (
            tc.tile_pool(name="data", bufs=N_GROUPS) as data,
            tc.tile_pool(name="gbuf", bufs=16) as gbuf,
            tc.tile_pool(name="work", bufs=8) as work,
            tc.tile_pool(name="single", bufs=1) as single,
            tc.tile_pool(name="psum", bufs=1, space="PSUM") as psum,
        ):
            lab_all = single.tile([P, N_TILES], i32)
            nc.sync.dma_start(out=lab_all[:], in_=lab_d[:, :])

            # x group-DMAs staggered between gathers (group g issued just
            # before gather 8g) so the SWDGE ring's SDMA consumption isn't
            # starved by a 4MB x flood at kernel start
            x_tiles = [None] * N_GROUPS

            def load_x_group(g):
                x_t = data.tile([P, GROUP, FEATURE_DIM], f32, tag="x")
                nc.sync.dma_start(
                    out=x_t[:],
                    in_=x_d[g * GROUP * P : (g + 1) * GROUP * P, :].rearrange(
                        "(t p) e -> p t e", p=P
                    ),
                )
                x_tiles[g] = x_t

            load_x_group(0)

            acc = single.tile([P, N_TILES], f32)
            for t in range(N_TILES):
                g, j = divmod(t, GROUP)
                if j == 0 and g + 1 < N_GROUPS and x_tiles[g + 1] is None:
                    load_x_group(g + 1)
                g_t = gbuf.tile([P, FEATURE_DIM], f32, tag="g")
                nc.gpsimd.indirect_dma_start(
                    out=g_t[:],
                    out_offset=None,
                    in_=cen_d[:, :],
                    in_offset=bass.IndirectOffsetOnAxis(
                        ap=lab_all[:, t : t + 1], axis=0
                    ),
                )
                d_t = work.tile([P, FEATURE_DIM], f32, tag="d")
                nc.vector.tensor_tensor(
                    out=d_t[:],
                    in0=x_tiles[g][:, j, :],
                    in1=g_t[:],
                    op=mybir.AluOpType.subtract,
                )
                s_t = work.tile([P, FEATURE_DIM], f32, tag="s")
                nc.scalar.activation(
                    out=s_t[:],
                    in_=d_t[:],
                    func=mybir.ActivationFunctionType.Square,
                    accum_out=acc[:, t : t + 1],
                )

            clipped = single.tile([P, N_TILES], f32)
            nc.vector.tensor_scalar(
                out=clipped[:],
                in0=acc[:],
                scalar1=float(CLAMP_MIN),
                scalar2=float(CLAMP_MAX),
                op0=mybir.AluOpType.max,
                op1=mybir.AluOpType.min,
            )
            rowsum = single.tile([P, 1], f32)
            nc.vector.reduce_sum(out=rowsum[:], in_=clipped[:], axis=mybir.AxisListType.X)

            ones = single.tile([P, 1], f32)
            nc.vector.memset(ones[:], 1.0)
            tot = psum.tile([1, 1], f32, space="PSUM")
            nc.tensor.matmul(out=tot[:], lhsT=rowsum[:], rhs=ones[:], start=True, stop=True)
            res = single.tile([1, 1], f32)
            nc.vector.tensor_copy(out=res[:], in_=tot[:])
            nc.sync.dma_start(out=out_d[:, :], in_=res[:])

    nc.finalize()
    return nc


def kernel(x: np.ndarray, centers: np.ndarray, labels: np.ndarray) -> np.ndarray:
    from concourse import bass_utils

    if "nc" not in _CACHE:
        _CACHE["nc"] = _build_nc()
    nc = _CACHE["nc"]

    x = np.ascontiguousarray(np.asarray(x, dtype=np.float32))
    centers = np.ascontiguousarray(np.asarray(centers, dtype=np.float32))
    lab = np.asarray(labels).astype(np.int64).reshape(N_CORES, N_TILES, P)

    xs = x.reshape(N_CORES, SHARD, FEATURE_DIM)
    in_maps = [
        {
            "x": np.ascontiguousarray(xs[c]),
            "labels": np.ascontiguousarray(lab[c].transpose(1, 0).astype(np.int32)),
            "centers": centers,
        }
        for c in range(N_CORES)
    ]

    rr = bass_utils.run_bass_kernel_spmd(nc, in_maps, list(range(N_CORES)))
    _CACHE["last_results"] = rr

    total = sum(float(r["out"][0, 0]) for r in rr.results)
    loss = (total + BATCH * (NUM_CLASSES - 1) * CLAMP_MIN) / BATCH
    return np.asarray(loss, dtype=np.float32)
